# revision 17
# baseline (speedup 1.0000x reference)
"""NTM head addressing kernel for Trainium2 (8 NeuronCores, data-parallel over heads).

Shapes (hardcoded): B=4096 heads, N=2048 memory rows, C=128 memory cols.
Each core processes 512 heads as 4 tiles of 128 (partition dim = head).

Math restructuring vs the reference (exact up to fp rounding):
  - w = w_tilde^gamma / sum(w_tilde^gamma) is invariant to any per-head
    positive scale on w_tilde.  We therefore drop the softmax normalizer of
    s (divide taps by s1) and the (1-g) factor of the interpolation:
        u      = (g/(1-g)/sum_e) * e + w_prev          (e = exp(beta*sim))
        v      = (s0/s1)*u_{j-1} + u_j + (s2/s1)*u_{j+1}   (circular)
        w      = v^gamma / sum(v^gamma)
    with g/(1-g) = exp(g_raw) (sigmoid odds), s0/s1 = exp(s0_raw-s1_raw).
  - beta' = softplus(beta_raw)/||k|| is applied as the per-partition scale
    of the ACT exp pass reading the matmul PSUM; sum_e comes free via
    accum_out.  Likewise gamma' scales the final exp (accum_out=sum_y).
  - EPS terms are dropped: |denominators| >= ~2.5e-14 >> 1e-16 always.

Layout/engine choices:
  - All activation functions pinned to the natural_log_exp_and_others table
    set -> no ACT table thrash.
  - Cosine-sim matmul in bf16 (fp32 PSUM accumulate).  The row-normalized
    M^T is built through a quarter-granular pipeline (quarter M DMA -> DVE
    square+rowsum -> rsqrt -> per-tile row scale -> PE transpose (bf16) ->
    PSUM->SBUF copy) so the first matmul starts ~10us in.
  - DVE: 3 scalar_tensor_tensor slab passes per tile + circular-edge STTs
    + final y/sum_y scale.  ACT: exp/ln/exp passes with fused row sums.
"""

import os
import numpy as np

_B, _N, _C = 4096, 2048, 128
_NCORES = 8
_BS = _B // _NCORES      # 512 heads per core
_NT = _BS // 128         # 4 head tiles per core
_MT = _N // 128          # 16 memory-row tiles
_NQ = 4                  # M quarters
_MTQ = _MT // _NQ

_MM_BF16 = os.environ.get("NTM_MM_BF16", "1") == "1"

_built = None

_ONE_SET = "natural_log_exp_and_others"
_PINNED = {"Exp", "Ln", "Square", "Copy", "Identity"}


def _patch_act_tables():
    """Force Exp/Ln/Square/Copy onto the one table set that holds them all,
    so bacc's load inserter cannot thrash between per-function sets."""
    import concourse.bacc as bacc
    import concourse.hw_specs as hw_specs
    import concourse.mybir as mybir

    if getattr(bacc, "_ntm_table_patch", False):
        return
    orig = hw_specs.get_activation_tables
    pinned = {
        getattr(mybir.ActivationFunctionType, n)
        for n in _PINNED
        if hasattr(mybir.ActivationFunctionType, n)
    }

    def patched(module_arch):
        tables = orig(module_arch)
        out = {}
        for name, fns in tables.items():
            if name != _ONE_SET:
                fns = fns - pinned
            out[name] = fns
        return out

    bacc.get_activation_tables = patched
    bacc._ntm_table_patch = True


def _build():
    """Construct the (SPMD, per-core) Bass program."""
    import concourse.bass as bass
    import concourse.bacc as bacc
    import concourse.mybir as mybir
    import concourse.tile as tile

    _patch_act_tables()

    f32 = mybir.dt.float32
    bf16 = mybir.dt.bfloat16
    mmdt = bf16 if _MM_BF16 else f32
    AF = mybir.ActivationFunctionType
    OP = mybir.AluOpType
    X = mybir.AxisListType.X

    nc = bacc.Bacc(
        "TRN2", target_bir_lowering=False, debug=False, num_devices=_NCORES
    )
    kT_d = nc.declare_dram_parameter("kT", [_C, _BS], mmdt, isOutput=False)
    kR_d = nc.declare_dram_parameter("kR", [_BS, _C], f32, isOutput=False)
    sm_d = nc.declare_dram_parameter("sm", [128, _NT * 6], f32, isOutput=False)
    wp_d = nc.declare_dram_parameter("wp", [_BS, _N], f32, isOutput=False)
    M_d = nc.declare_dram_parameter("M", [_N, _C], f32, isOutput=False)
    eye_d = nc.declare_dram_parameter("eye", [128, 128], mmdt, isOutput=False)
    out_d = nc.declare_dram_parameter("out", [_BS, _N], f32, isOutput=True)

    with tile.TileContext(nc) as tc:
        with (
            tc.tile_pool(name="const", bufs=1) as constp,
            tc.tile_pool(name="setup", bufs=1) as setupp,
            tc.tile_pool(name="slab", bufs=2) as slabp,
            tc.tile_pool(name="mini", bufs=2) as minip,
            tc.tile_pool(name="psum", bufs=2, space=bass.MemorySpace.PSUM) as psump,
        ):
            # ---------------- setup: loads (M quarters first) ------------
            kR = constp.tile([128, _NT, _C], f32)
            nc.sync.dma_start(kR[:], kR_d[:].rearrange("(t p) c -> p t c", p=128))
            sm = constp.tile([128, _NT * 6], f32)
            nc.sync.dma_start(sm[:], sm_d[:])
            kT = constp.tile([_C, _BS], mmdt)
            nc.sync.dma_start(kT[:], kT_d[:])
            eye = constp.tile([128, 128], mmdt)
            nc.sync.dma_start(eye[:], eye_d[:])
            mq = []
            for q in range(_NQ):
                mq.append(
                    constp.tile([128, _MTQ, _C], f32, tag=f"mq{q}", name=f"mq{q}")
                )
                nc.sync.dma_start(
                    mq[q][:],
                    M_d[:].rearrange("(t p) c -> p t c", p=128)[
                        :, q * _MTQ : (q + 1) * _MTQ, :
                    ],
                )

            # --------- per-head scalars FIRST (gates the first exp) -------
            ksqs = setupp.tile([128, _NT, _C], f32)
            nc.vector.tensor_mul(ksqs[:], kR[:], kR[:])
            ksq = minip.tile([128, _NT], f32, tag="ksq")
            nc.vector.tensor_reduce(ksq[:], ksqs[:], X, OP.add)
            lnksq = minip.tile([128, _NT], f32, tag="lnksq")
            nc.scalar.activation(lnksq[:], ksq[:], AF.Ln)
            rk = minip.tile([128, _NT], f32, tag="rk")
            nc.scalar.activation(rk[:], lnksq[:], AF.Exp, scale=-0.5)
            be = minip.tile([128, _NT], f32, tag="be")
            nc.scalar.activation(be[:], sm[:, 0:_NT], AF.Exp)
            bsp = minip.tile([128, _NT], f32, tag="bsp")
            nc.scalar.activation(bsp[:], be[:], AF.Ln, bias=1.0)
            bprime = minip.tile([128, _NT], f32, tag="bprime")
            nc.vector.tensor_mul(bprime[:], bsp[:], rk[:])
            eg = minip.tile([128, _NT], f32, tag="eg")
            nc.scalar.activation(eg[:], sm[:, _NT : 2 * _NT], AF.Exp)
            ge = minip.tile([128, _NT], f32, tag="ge")
            nc.scalar.activation(ge[:], sm[:, 2 * _NT : 3 * _NT], AF.Exp)
            gsp = minip.tile([128, _NT], f32, tag="gsp")
            nc.scalar.activation(gsp[:], ge[:], AF.Ln, bias=1.0)
            gprime = minip.tile([128, _NT], f32, tag="gprime")
            nc.vector.tensor_scalar_add(gprime[:], gsp[:], 1.0)
            d02 = minip.tile([128, 2 * _NT], f32, tag="d02")
            nc.vector.tensor_sub(d02[:, 0:_NT], sm[:, 3 * _NT : 4 * _NT], sm[:, 4 * _NT : 5 * _NT])
            nc.vector.tensor_sub(d02[:, _NT : 2 * _NT], sm[:, 5 * _NT : 6 * _NT], sm[:, 4 * _NT : 5 * _NT])
            s02 = minip.tile([128, 2 * _NT], f32, tag="s02")
            nc.scalar.activation(s02[:], d02[:], AF.Exp)

            # ------- setup: M row norms -> normalized M^T, per quarter ----
            MThq = []
            for q in range(_NQ):
                MThq.append(
                    constp.tile(
                        [_C, _MTQ * 128], mmdt, tag=f"MTh{q}", name=f"MTh{q}"
                    )
                )
            # tile-0 sim is interleaved with the M-prep quarters below so
            # its softmax starts as soon as each quarter of M^T lands.
            logits0 = psump.tile([128, _N], f32, tag="ps", name="logits0")
            e0 = slabp.tile([128, _N], f32, tag="e", name="e0")
            seq0 = minip.tile([128, _NQ], f32, tag="seq", name="seq0")
            for q in range(_NQ):
                msqs = setupp.tile([128, _MTQ, _C], f32, tag=f"msqs{q % 2}", name=f"msqs{q}")
                nc.vector.tensor_mul(msqs[:], mq[q][:], mq[q][:])
                msq = minip.tile([128, _MTQ], f32, tag=f"msq{q}", name=f"msq{q}")
                nc.vector.tensor_reduce(msq[:], msqs[:], X, OP.add)
                # rmn = msq^-0.5 = exp(-0.5*ln(msq))  (ACT Rsqrt is banned)
                lnm = minip.tile([128, _MTQ], f32, tag=f"lnm{q}", name=f"lnm{q}")
                nc.scalar.activation(lnm[:], msq[:], AF.Ln)
                rmn = minip.tile([128, _MTQ], f32, tag=f"rmn{q}", name=f"rmn{q}")
                nc.scalar.activation(rmn[:], lnm[:], AF.Exp, scale=-0.5)
                trq = psump.tile([128, _MTQ * 128], mmdt, tag="ps", name=f"trq{q}")
                for i in range(_MTQ):
                    mt = q * _MTQ + i
                    mh = setupp.tile(
                        [128, _C], mmdt, tag=f"mh{mt % 4}", name=f"mh{mt}", bufs=2
                    )
                    if i == 3:
                        nc.scalar.mul(mh[:], mq[q][:, i, :], rmn[:, i : i + 1])
                    else:
                        nc.vector.tensor_scalar_mul(
                            mh[:], mq[q][:, i, :], rmn[:, i : i + 1]
                        )
                    nc.tensor.transpose(
                        trq[:, i * 128 : (i + 1) * 128], mh[:], eye[:]
                    )
                # one batched PSUM->SBUF copy per quarter
                nc.scalar.copy(MThq[q][:], trq[:])
                # tile-0 matmul + exp for this quarter right away
                nc.tensor.matmul(
                    logits0[:, q * 512 : (q + 1) * 512],
                    kT[:, 0:128],
                    MThq[q][:],
                )
                nc.scalar.activation(
                    e0[:, q * 512 : (q + 1) * 512],
                    logits0[:, q * 512 : (q + 1) * 512],
                    AF.Exp,
                    scale=bprime[:, 0:1],
                    accum_out=seq0[:, q : q + 1],
                )

            # ---------------- main loop over 4 head tiles ----------------
            for t in range(_NT):
                wp = slabp.tile([128, _N], f32, tag="wp", bufs=3)
                nc.sync.dma_start(wp[:], wp_d[:][t * 128 : (t + 1) * 128, :])

                sume = minip.tile([128, 1], f32, tag="sume")
                if t == 0:
                    e = e0
                    sduo = minip.tile([128, 2], f32, tag="sduo")
                    nc.vector.tensor_add(sduo[:, 0:1], seq0[:, 0:1], seq0[:, 1:2])
                    nc.vector.tensor_add(sduo[:, 1:2], seq0[:, 2:3], seq0[:, 3:4])
                    nc.vector.tensor_add(sume[:], sduo[:, 0:1], sduo[:, 1:2])
                else:
                    logits = psump.tile([128, _N], f32, tag="ps")
                    for q in range(_NQ):
                        nc.tensor.matmul(
                            logits[:, q * 512 : (q + 1) * 512],
                            kT[:, t * 128 : (t + 1) * 128],
                            MThq[q][:],
                        )
                    e = slabp.tile([128, _N], f32, tag="e")
                    nc.scalar.activation(
                        e[:], logits[:], AF.Exp,
                        scale=bprime[:, t : t + 1], accum_out=sume[:],
                    )

                # a = eg / sum_e
                rse = minip.tile([128, 1], f32, tag="rse")
                nc.vector.reciprocal(rse[:], sume[:])
                a = minip.tile([128, 1], f32, tag="a")
                nc.vector.tensor_mul(a[:], eg[:, t : t + 1], rse[:])

                # u = a*e + w_prev, written into a circularly padded buffer:
                # pad[:, 1+j] = u[j]; pad[:, 0] = u[N-1]; pad[:, N+1] = u[0]
                pad = slabp.tile([128, _N + 2], f32, tag="pad")
                nc.vector.scalar_tensor_tensor(
                    pad[:, 1 : _N + 1], e[:], a[:], wp[:], OP.mult, OP.add
                )
                nc.scalar.copy(pad[:, 0:1], pad[:, _N : _N + 1])
                nc.scalar.copy(pad[:, _N + 1 : _N + 2], pad[:, 1:2])

                # circular 3-tap conv (middle tap normalized to 1), all
                # full-width:  c = s0'*u_{j-1} + u_j ;  v = s2'*u_{j+1} + c
                s0a = s02[:, t : t + 1]
                s2a = s02[:, _NT + t : _NT + t + 1]
                c = slabp.tile([128, _N], f32, tag="c")
                nc.vector.scalar_tensor_tensor(
                    c[:], pad[:, 0:_N], s0a, pad[:, 1 : _N + 1], OP.mult, OP.add
                )
                v = slabp.tile([128, _N], f32, tag="v")
                nc.vector.scalar_tensor_tensor(
                    v[:], pad[:, 2 : _N + 2], s2a, c[:], OP.mult, OP.add
                )

                # sharpen: y = v^gamma' = exp(gamma' * ln v), sum_y fused
                lw = slabp.tile([128, _N], f32, tag="lw")
                nc.scalar.activation(lw[:], v[:], AF.Ln)
                y = slabp.tile([128, _N], f32, tag="y")
                sumy = minip.tile([128, 1], f32, tag="sumy")
                nc.scalar.activation(
                    y[:], lw[:], AF.Exp,
                    scale=gprime[:, t : t + 1], accum_out=sumy[:],
                )

                # w = y / sum_y  (half-width passes; each half DMAs out
                # as soon as it is scaled; engine alternates per tile)
                r = minip.tile([128, 1], f32, tag="r")
                nc.vector.reciprocal(r[:], sumy[:])
                wout = slabp.tile([128, _N], f32, tag="wout")
                nh = 4 if t == _NT - 1 else 2
                H = _N // nh
                for h in range(nh):
                    sl = slice(h * H, (h + 1) * H)
                    if t == 0:
                        nc.scalar.mul(wout[:, sl], y[:, sl], r[:])
                    else:
                        nc.vector.tensor_scalar_mul(wout[:, sl], y[:, sl], r[:])
                    nc.sync.dma_start(
                        out_d[:][t * 128 : (t + 1) * 128, sl], wout[:, sl]
                    )

    nc.compile()
    return nc


def _get_nc():
    global _built
    if _built is None:
        _built = _build()
    return _built


def _make_in_maps(k, beta, g, s, gamma, w_prev, M):
    import ml_dtypes

    mmdt = ml_dtypes.bfloat16 if _MM_BF16 else np.float32
    eye = np.eye(128, dtype=mmdt)
    Mc = np.ascontiguousarray(M, dtype=np.float32)
    in_maps = []
    for c in range(_NCORES):
        sl = slice(c * _BS, (c + 1) * _BS)
        ks = np.ascontiguousarray(k[sl], dtype=np.float32)          # [512,128]
        kTs = np.ascontiguousarray(ks.T.astype(mmdt))               # [128,512]
        # packed per-head scalars: [128, 6*NT]; col block order:
        # beta, g, gamma, s0, s1, s2 (each NT wide; head = t*128 + p)
        def cols(x):
            return np.ascontiguousarray(x.reshape(_NT, 128).T, dtype=np.float32)
        sm = np.concatenate(
            [
                cols(beta[sl, 0]),
                cols(g[sl, 0]),
                cols(gamma[sl, 0]),
                cols(s[sl, 0]),
                cols(s[sl, 1]),
                cols(s[sl, 2]),
            ],
            axis=1,
        )
        in_maps.append(
            {
                "kT": kTs,
                "kR": ks,
                "sm": np.ascontiguousarray(sm),
                "wp": np.ascontiguousarray(w_prev[sl], dtype=np.float32),
                "M": Mc,
                "eye": eye,
            }
        )
    return in_maps


def kernel(k, beta, g, s, gamma, w_prev, M, _trace=False, _tmpdir=None):
    from concourse.bass_utils import run_bass_kernel_spmd

    nc = _get_nc()
    in_maps = _make_in_maps(
        np.asarray(k), np.asarray(beta), np.asarray(g), np.asarray(s),
        np.asarray(gamma), np.asarray(w_prev), np.asarray(M),
    )
    res = run_bass_kernel_spmd(
        nc, in_maps, list(range(_NCORES)), trace=_trace, tmpdir=_tmpdir
    )
    out = np.concatenate([res.results[c]["out"] for c in range(_NCORES)], axis=0)
    if _trace:
        kernel._last_results = res
    return out


# revision 19
# speedup vs baseline: 1.0098x; 1.0098x over previous
"""NTM head addressing kernel for Trainium2 (8 NeuronCores, data-parallel over heads).

Shapes (hardcoded): B=4096 heads, N=2048 memory rows, C=128 memory cols.
Each core processes 512 heads as 4 tiles of 128 (partition dim = head).

Math restructuring vs the reference (exact up to fp rounding):
  - w = w_tilde^gamma / sum(w_tilde^gamma) is invariant to any per-head
    positive scale on w_tilde.  We therefore drop the softmax normalizer of
    s (divide taps by s1) and the (1-g) factor of the interpolation:
        u      = (g/(1-g)/sum_e) * e + w_prev          (e = exp(beta*sim))
        v      = (s0/s1)*u_{j-1} + u_j + (s2/s1)*u_{j+1}   (circular)
        w      = v^gamma / sum(v^gamma)
    with g/(1-g) = exp(g_raw) (sigmoid odds), s0/s1 = exp(s0_raw-s1_raw).
  - beta' = softplus(beta_raw)/||k|| is applied as the per-partition scale
    of the ACT exp pass reading the matmul PSUM; sum_e comes free via
    accum_out.  Likewise gamma' scales the final exp (accum_out=sum_y).
  - EPS terms are dropped: |denominators| >= ~2.5e-14 >> 1e-16 always.

Layout/engine choices:
  - All activation functions pinned to the natural_log_exp_and_others table
    set -> no ACT table thrash.
  - Cosine-sim matmul in bf16 (fp32 PSUM accumulate).  The row-normalized
    M^T is built through a quarter-granular pipeline (quarter M DMA -> DVE
    square+rowsum -> rsqrt -> per-tile row scale -> PE transpose (bf16) ->
    PSUM->SBUF copy) so the first matmul starts ~10us in.
  - DVE: 3 scalar_tensor_tensor slab passes per tile + circular-edge STTs
    + final y/sum_y scale.  ACT: exp/ln/exp passes with fused row sums.
"""

import os
import numpy as np

_B, _N, _C = 4096, 2048, 128
_NCORES = 8
_BS = _B // _NCORES      # 512 heads per core
_NT = _BS // 128         # 4 head tiles per core
_MT = _N // 128          # 16 memory-row tiles
_NQ = 4                  # M quarters
_MTQ = _MT // _NQ

_MM_BF16 = os.environ.get("NTM_MM_BF16", "1") == "1"

_built = None

_ONE_SET = "natural_log_exp_and_others"
_PINNED = {"Exp", "Ln", "Square", "Copy", "Identity"}


def _patch_act_tables():
    """Force Exp/Ln/Square/Copy onto the one table set that holds them all,
    so bacc's load inserter cannot thrash between per-function sets."""
    import concourse.bacc as bacc
    import concourse.hw_specs as hw_specs
    import concourse.mybir as mybir

    if getattr(bacc, "_ntm_table_patch", False):
        return
    orig = hw_specs.get_activation_tables
    pinned = {
        getattr(mybir.ActivationFunctionType, n)
        for n in _PINNED
        if hasattr(mybir.ActivationFunctionType, n)
    }

    def patched(module_arch):
        tables = orig(module_arch)
        out = {}
        for name, fns in tables.items():
            if name != _ONE_SET:
                fns = fns - pinned
            out[name] = fns
        return out

    bacc.get_activation_tables = patched
    bacc._ntm_table_patch = True


def _build():
    """Construct the (SPMD, per-core) Bass program."""
    import concourse.bass as bass
    import concourse.bacc as bacc
    import concourse.mybir as mybir
    import concourse.tile as tile

    _patch_act_tables()

    f32 = mybir.dt.float32
    bf16 = mybir.dt.bfloat16
    mmdt = bf16 if _MM_BF16 else f32
    AF = mybir.ActivationFunctionType
    OP = mybir.AluOpType
    X = mybir.AxisListType.X

    nc = bacc.Bacc(
        "TRN2", target_bir_lowering=False, debug=False, num_devices=_NCORES
    )
    kT_d = nc.declare_dram_parameter("kT", [_C, _BS], mmdt, isOutput=False)
    kR_d = nc.declare_dram_parameter("kR", [_BS, _C], f32, isOutput=False)
    sm_d = nc.declare_dram_parameter("sm", [128, _NT * 6], f32, isOutput=False)
    wp_d = nc.declare_dram_parameter("wp", [_BS, _N], f32, isOutput=False)
    M_d = nc.declare_dram_parameter("M", [_N, _C], f32, isOutput=False)
    eye_d = nc.declare_dram_parameter("eye", [128, 128], mmdt, isOutput=False)
    out_d = nc.declare_dram_parameter("out", [_BS, _N], f32, isOutput=True)

    with tile.TileContext(nc) as tc:
        with (
            tc.tile_pool(name="const", bufs=1) as constp,
            tc.tile_pool(name="setup", bufs=1) as setupp,
            tc.tile_pool(name="slab", bufs=2) as slabp,
            tc.tile_pool(name="mini", bufs=2) as minip,
            tc.tile_pool(name="psum", bufs=2, space=bass.MemorySpace.PSUM) as psump,
        ):
            # ---------------- setup: loads (M quarters first) ------------
            kR = constp.tile([128, _NT, _C], f32)
            nc.sync.dma_start(kR[:], kR_d[:].rearrange("(t p) c -> p t c", p=128))
            sm = constp.tile([128, _NT * 6], f32)
            nc.sync.dma_start(sm[:], sm_d[:])
            kT = constp.tile([_C, _BS], mmdt)
            nc.sync.dma_start(kT[:], kT_d[:])
            mq = []
            for q in range(_NQ):
                mq.append(
                    constp.tile([128, _MTQ, _C], f32, tag=f"mq{q}", name=f"mq{q}")
                )
                nc.sync.dma_start(
                    mq[q][:],
                    M_d[:].rearrange("(t p) c -> p t c", p=128)[
                        :, q * _MTQ : (q + 1) * _MTQ, :
                    ],
                )
            eye = constp.tile([128, 128], mmdt)
            nc.sync.dma_start(eye[:], eye_d[:])

            # --------- per-head scalars FIRST (gates the first exp) -------
            ksqs = setupp.tile([128, _NT, _C], f32)
            nc.vector.tensor_mul(ksqs[:], kR[:], kR[:])
            ksq = minip.tile([128, _NT], f32, tag="ksq")
            nc.vector.tensor_reduce(ksq[:], ksqs[:], X, OP.add)
            lnksq = minip.tile([128, _NT], f32, tag="lnksq")
            nc.scalar.activation(lnksq[:], ksq[:], AF.Ln)
            rk = minip.tile([128, _NT], f32, tag="rk")
            nc.scalar.activation(rk[:], lnksq[:], AF.Exp, scale=-0.5)
            be = minip.tile([128, _NT], f32, tag="be")
            nc.scalar.activation(be[:], sm[:, 0:_NT], AF.Exp)
            bsp = minip.tile([128, _NT], f32, tag="bsp")
            nc.scalar.activation(bsp[:], be[:], AF.Ln, bias=1.0)
            bprime = minip.tile([128, _NT], f32, tag="bprime")
            nc.vector.tensor_mul(bprime[:], bsp[:], rk[:])
            eg = minip.tile([128, _NT], f32, tag="eg")
            nc.scalar.activation(eg[:], sm[:, _NT : 2 * _NT], AF.Exp)
            ge = minip.tile([128, _NT], f32, tag="ge")
            nc.scalar.activation(ge[:], sm[:, 2 * _NT : 3 * _NT], AF.Exp)
            gsp = minip.tile([128, _NT], f32, tag="gsp")
            nc.scalar.activation(gsp[:], ge[:], AF.Ln, bias=1.0)
            gprime = minip.tile([128, _NT], f32, tag="gprime")
            nc.vector.tensor_scalar_add(gprime[:], gsp[:], 1.0)
            d02 = minip.tile([128, 2 * _NT], f32, tag="d02")
            nc.vector.tensor_sub(d02[:, 0:_NT], sm[:, 3 * _NT : 4 * _NT], sm[:, 4 * _NT : 5 * _NT])
            nc.vector.tensor_sub(d02[:, _NT : 2 * _NT], sm[:, 5 * _NT : 6 * _NT], sm[:, 4 * _NT : 5 * _NT])
            s02 = minip.tile([128, 2 * _NT], f32, tag="s02")
            nc.scalar.activation(s02[:], d02[:], AF.Exp)

            # ------- setup: M row norms -> normalized M^T, per quarter ----
            MThq = []
            for q in range(_NQ):
                MThq.append(
                    constp.tile(
                        [_C, _MTQ * 128], mmdt, tag=f"MTh{q}", name=f"MTh{q}"
                    )
                )
            # tile-0 sim is interleaved with the M-prep quarters below so
            # its softmax starts as soon as each quarter of M^T lands.
            logits0 = psump.tile([128, _N], f32, tag="ps", name="logits0")
            e0 = slabp.tile([128, _N], f32, tag="e", name="e0")
            seq0 = minip.tile([128, _NQ], f32, tag="seq", name="seq0")
            for q in range(_NQ):
                msqs = setupp.tile([128, _MTQ, _C], f32, tag=f"msqs{q % 2}", name=f"msqs{q}")
                nc.vector.tensor_mul(msqs[:], mq[q][:], mq[q][:])
                msq = minip.tile([128, _MTQ], f32, tag=f"msq{q}", name=f"msq{q}")
                nc.vector.tensor_reduce(msq[:], msqs[:], X, OP.add)
                # rmn = msq^-0.5 = exp(-0.5*ln(msq))  (ACT Rsqrt is banned)
                lnm = minip.tile([128, _MTQ], f32, tag=f"lnm{q}", name=f"lnm{q}")
                nc.scalar.activation(lnm[:], msq[:], AF.Ln)
                rmn = minip.tile([128, _MTQ], f32, tag=f"rmn{q}", name=f"rmn{q}")
                nc.scalar.activation(rmn[:], lnm[:], AF.Exp, scale=-0.5)
                trq = psump.tile([128, _MTQ * 128], mmdt, tag="ps", name=f"trq{q}")
                for i in range(_MTQ):
                    mt = q * _MTQ + i
                    mh = setupp.tile(
                        [128, _C], mmdt, tag=f"mh{mt % 4}", name=f"mh{mt}", bufs=2
                    )
                    nc.vector.tensor_scalar_mul(
                        mh[:], mq[q][:, i, :], rmn[:, i : i + 1]
                    )
                    nc.tensor.transpose(
                        trq[:, i * 128 : (i + 1) * 128], mh[:], eye[:]
                    )
                # one batched PSUM->SBUF copy per quarter
                nc.scalar.copy(MThq[q][:], trq[:])
                # tile-0 matmul + exp for this quarter right away
                nc.tensor.matmul(
                    logits0[:, q * 512 : (q + 1) * 512],
                    kT[:, 0:128],
                    MThq[q][:],
                )
                nc.scalar.activation(
                    e0[:, q * 512 : (q + 1) * 512],
                    logits0[:, q * 512 : (q + 1) * 512],
                    AF.Exp,
                    scale=bprime[:, 0:1],
                    accum_out=seq0[:, q : q + 1],
                )

            # ---------------- main loop over 4 head tiles ----------------
            deferred = []

            def emit_final(t, y, sumy):
                # w = y / sum_y; last tile streams out in quarters
                r = minip.tile([128, 1], f32, tag="r", name=f"r{t}")
                nc.vector.reciprocal(r[:], sumy[:])
                wout = slabp.tile([128, _N], f32, tag="wout", name=f"wout{t}")
                nh = 4 if t == _NT - 1 else 2
                H = _N // nh
                for h in range(nh):
                    sl = slice(h * H, (h + 1) * H)
                    if t == 0:
                        nc.scalar.mul(wout[:, sl], y[:, sl], r[:])
                    else:
                        nc.vector.tensor_scalar_mul(wout[:, sl], y[:, sl], r[:])
                    nc.sync.dma_start(
                        out_d[:][t * 128 : (t + 1) * 128, sl], wout[:, sl]
                    )

            for t in range(_NT):
                wp = slabp.tile([128, _N], f32, tag="wp", bufs=3)
                nc.sync.dma_start(wp[:], wp_d[:][t * 128 : (t + 1) * 128, :])

                sume = minip.tile([128, 1], f32, tag="sume")
                if t == 0:
                    e = e0
                    sduo = minip.tile([128, 2], f32, tag="sduo")
                    nc.vector.tensor_add(sduo[:, 0:1], seq0[:, 0:1], seq0[:, 1:2])
                    nc.vector.tensor_add(sduo[:, 1:2], seq0[:, 2:3], seq0[:, 3:4])
                    nc.vector.tensor_add(sume[:], sduo[:, 0:1], sduo[:, 1:2])
                else:
                    logits = psump.tile([128, _N], f32, tag="ps")
                    for q in range(_NQ):
                        nc.tensor.matmul(
                            logits[:, q * 512 : (q + 1) * 512],
                            kT[:, t * 128 : (t + 1) * 128],
                            MThq[q][:],
                        )
                    e = slabp.tile([128, _N], f32, tag="e")
                    nc.scalar.activation(
                        e[:], logits[:], AF.Exp,
                        scale=bprime[:, t : t + 1], accum_out=sume[:],
                    )

                # a = eg / sum_e
                rse = minip.tile([128, 1], f32, tag="rse")
                nc.vector.reciprocal(rse[:], sume[:])
                a = minip.tile([128, 1], f32, tag="a")
                nc.vector.tensor_mul(a[:], eg[:, t : t + 1], rse[:])

                # u = a*e + w_prev, written into a circularly padded buffer:
                # pad[:, 1+j] = u[j]; pad[:, 0] = u[N-1]; pad[:, N+1] = u[0]
                pad = slabp.tile([128, _N + 2], f32, tag="pad")
                nc.vector.scalar_tensor_tensor(
                    pad[:, 1 : _N + 1], e[:], a[:], wp[:], OP.mult, OP.add
                )
                nc.scalar.copy(pad[:, 0:1], pad[:, _N : _N + 1])
                nc.scalar.copy(pad[:, _N + 1 : _N + 2], pad[:, 1:2])

                # circular 3-tap conv (middle tap normalized to 1), all
                # full-width:  c = s0'*u_{j-1} + u_j ;  v = s2'*u_{j+1} + c
                s0a = s02[:, t : t + 1]
                s2a = s02[:, _NT + t : _NT + t + 1]
                c = slabp.tile([128, _N], f32, tag="c")
                nc.vector.scalar_tensor_tensor(
                    c[:], pad[:, 0:_N], s0a, pad[:, 1 : _N + 1], OP.mult, OP.add
                )
                v = slabp.tile([128, _N], f32, tag="v")
                nc.vector.scalar_tensor_tensor(
                    v[:], pad[:, 2 : _N + 2], s2a, c[:], OP.mult, OP.add
                )

                if t == _NT - 1:
                    for (dt_, dy_, ds_) in deferred:
                        emit_final(dt_, dy_, ds_)

                # sharpen: y = v^gamma' = exp(gamma' * ln v), sum_y fused
                lw = slabp.tile([128, _N], f32, tag="lw")
                nc.scalar.activation(lw[:], v[:], AF.Ln)
                y = slabp.tile([128, _N], f32, tag="y")
                sumy = minip.tile([128, 1], f32, tag="sumy")
                nc.scalar.activation(
                    y[:], lw[:], AF.Exp,
                    scale=gprime[:, t : t + 1], accum_out=sumy[:],
                )

                if t in (1, 2):
                    deferred.append((t, y, sumy))
                else:
                    emit_final(t, y, sumy)

    nc.compile()
    return nc


def _get_nc():
    global _built
    if _built is None:
        _built = _build()
    return _built


def _make_in_maps(k, beta, g, s, gamma, w_prev, M):
    import ml_dtypes

    mmdt = ml_dtypes.bfloat16 if _MM_BF16 else np.float32
    eye = np.eye(128, dtype=mmdt)
    Mc = np.ascontiguousarray(M, dtype=np.float32)
    in_maps = []
    for c in range(_NCORES):
        sl = slice(c * _BS, (c + 1) * _BS)
        ks = np.ascontiguousarray(k[sl], dtype=np.float32)          # [512,128]
        kTs = np.ascontiguousarray(ks.T.astype(mmdt))               # [128,512]
        # packed per-head scalars: [128, 6*NT]; col block order:
        # beta, g, gamma, s0, s1, s2 (each NT wide; head = t*128 + p)
        def cols(x):
            return np.ascontiguousarray(x.reshape(_NT, 128).T, dtype=np.float32)
        sm = np.concatenate(
            [
                cols(beta[sl, 0]),
                cols(g[sl, 0]),
                cols(gamma[sl, 0]),
                cols(s[sl, 0]),
                cols(s[sl, 1]),
                cols(s[sl, 2]),
            ],
            axis=1,
        )
        in_maps.append(
            {
                "kT": kTs,
                "kR": ks,
                "sm": np.ascontiguousarray(sm),
                "wp": np.ascontiguousarray(w_prev[sl], dtype=np.float32),
                "M": Mc,
                "eye": eye,
            }
        )
    return in_maps


def kernel(k, beta, g, s, gamma, w_prev, M, _trace=False, _tmpdir=None):
    from concourse.bass_utils import run_bass_kernel_spmd

    nc = _get_nc()
    in_maps = _make_in_maps(
        np.asarray(k), np.asarray(beta), np.asarray(g), np.asarray(s),
        np.asarray(gamma), np.asarray(w_prev), np.asarray(M),
    )
    res = run_bass_kernel_spmd(
        nc, in_maps, list(range(_NCORES)), trace=_trace, tmpdir=_tmpdir
    )
    out = np.concatenate([res.results[c]["out"] for c in range(_NCORES)], axis=0)
    if _trace:
        kernel._last_results = res
    return out
